# revision 16
# baseline (speedup 1.0000x reference)
"""Causal self-attention (RoPE, GQA) on 8 Trainium2 NeuronCores.

Sharding: 2-way data-parallel over batch x 4-way tensor-parallel over heads.
Core c handles batch c//4 and head-group c%4 (4 q-heads, 2 kv-heads).
Each core computes its partial output projection (wo row-shard); the host
sums the 4 partials per batch (the "all-reduce" happens in the unshard step).

v3 layout/scheduling notes (empirically driven from the v1/v2 traces):
  - Everything on device is bf16 (fp32r moving operands stream at ~2x the
    bf16 rate on HW and fp32 LDWEIGHTS at 395ns can't hide under matmuls).
  - DMA descriptors cost ~700ns of issue time on the triggering engine, so
    weights/x are loaded in 4-kt batches, not per-tile.
  - qt/kt/ot/v live in per-(head, q-chunk) tiles: the Tile framework tracks
    dependencies at tile granularity, so a full-S tile would make phase-2
    readers wait for the LAST q-chunk's RoPE write (7.8us false stall).
  - Attention iterates per 128-wide k-block: S matmul -> exp -> (diag:
    128x128 triangle mask only) -> PV + Z accumulation, scores transposed
    [k, q] so exp output feeds PV directly.
  - Z via a ones-vector matmul; 1/Z broadcast with a K=1 fp16 matmul (fp32
    broadcast matmuls cost 4 cycles/row).  The broadcast + normalize for
    head h are DEFERRED past the next head's first k-block so the in-order
    PE queue never waits on the DVE reciprocal chain.
  - Output-projection pairs (8 matmuls -> one 512KB DMA, queues alternated)
    are injected into the attention loop of the NEXT q-chunk.
  - Phase-1 PSUM drain is split across ACT and DVE; RoPE runs SBUF-side in
    bf16 on the DVE.
"""

import sys
import numpy as np
import ml_dtypes

sys.path.insert(0, "/opt/trn_rl_repo")

import concourse.bass as bass
import concourse.bacc as bacc
import concourse.mybir as mybir
from concourse import tile
from concourse.bass_utils import run_bass_kernel_spmd

F32 = mybir.dt.float32
BF16 = mybir.dt.bfloat16
FP16 = mybir.dt.float16
AF = mybir.ActivationFunctionType
OP = mybir.AluOpType

B, S, D = 2, 2048, 2048
HQ, HKV, HD = 16, 8, 128
ROPE_THETA = 10000.0
NCORES, TP = 8, 4
HQL, HKL = HQ // TP, HKV // TP        # 4 q heads, 2 kv heads per core
NKT = D // 128                        # 16 contraction tiles
QC = 512                              # q-chunk width
NQC = S // QC                         # 4 q chunks
NSB = S // 128                        # 16 s-blocks
SCALE = 1.0 / float(np.sqrt(HD))
BF = ml_dtypes.bfloat16


def _build_nc():
    nc = bacc.Bacc("TRN2", target_bir_lowering=False)

    xT_d = nc.dram_tensor("xT", [NKT, 128, S], BF16, kind="ExternalInput")
    wq_d = nc.dram_tensor("wq_t", [NKT, 128, HQL * HD], BF16, kind="ExternalInput")
    wk_d = nc.dram_tensor("wk_t", [NKT, 128, HKL * HD], BF16, kind="ExternalInput")
    wv_d = nc.dram_tensor("wv_t", [NKT, 128, HKL * HD], BF16, kind="ExternalInput")
    wo_d = nc.dram_tensor("wo_t", [HQL, 128, D], BF16, kind="ExternalInput")
    cos_d = nc.dram_tensor("cos128", [128, S], BF16, kind="ExternalInput")
    sin_d = nc.dram_tensor("sinM", [128, S], BF16, kind="ExternalInput")
    tri_d = nc.dram_tensor("tri", [128, 128], BF16, kind="ExternalInput")
    out_d = nc.dram_tensor("out", [NSB, 128, D], F32, kind="ExternalOutput")

    with tile.TileContext(nc) as tc:
        with (
            tc.tile_pool(name="resident", bufs=1) as res,
            tc.tile_pool(name="xstream", bufs=3) as xpool,
            tc.tile_pool(name="ropetmp", bufs=2) as rtmp,
            tc.tile_pool(name="evpool", bufs=8) as evpool,
            tc.tile_pool(name="epool", bufs=4) as epool,
            tc.tile_pool(name="small", bufs=2) as small,
            tc.tile_pool(name="outp", bufs=2) as outp,
        ):
            # ---------- resident tiles ----------
            wq_sb = res.tile([128, NKT, HQL * HD], BF16)
            wk_sb = res.tile([128, NKT, HKL * HD], BF16)
            wv_sb = res.tile([128, NKT, HKL * HD], BF16)
            wo_sb = res.tile([128, HQL, D], BF16)
            cos_sb = res.tile([128, S], BF16)
            sin_sb = res.tile([128, S], BF16)
            tri_sb = res.tile([128, 128], BF16)

            ones_bf = res.tile([128, 1], BF16)
            nc.vector.memset(ones_bf[:], 1.0)
            ones_h = res.tile([1, 128], FP16)
            nc.vector.memset(ones_h[:], 1.0)

            # phase-1 outputs, one tile per (head, q-chunk) so phase-2
            # readers never pick up false deps on later chunks' writes
            qt_sb = [[res.tile([128, QC], BF16, tag=f"qt{h}_{c}", name=f"qt{h}_{c}")
                      for c in range(NQC)] for h in range(HQL)]
            kt_sb = [[res.tile([128, QC], BF16, tag=f"kt{h}_{c}", name=f"kt{h}_{c}")
                      for c in range(NQC)] for h in range(HKL)]
            v_sb = [res.tile([128, 4, HKL * HD], BF16, tag=f"v{c}", name=f"v{c}")
                    for c in range(NQC)]
            ot_sb = [[res.tile([128, QC], BF16, tag=f"ot{h}_{c}", name=f"ot{h}_{c}")
                      for c in range(NQC)] for h in range(HQL)]

            rope_q = []               # deferred qc-3 rope closures

            # ---------- phase 1: QKV projection + RoPE ----------
            with tc.tile_pool(name="ps1", bufs=1, space="PSUM") as ps1:
                for qc in range(NQC):
                    qsl = slice(qc * QC, (qc + 1) * QC)
                    qps = [ps1.tile([128, QC], F32, tag=f"qps{h}", name=f"qps{h}_{qc}") for h in range(HQL)]
                    kps = [ps1.tile([128, QC], F32, tag=f"kps{h}", name=f"kps{h}_{qc}") for h in range(HKL)]
                    vps = ps1.tile([128, 4, HKL * HD], F32, tag="vps")
                    # smaller first batches so the first matmuls start early
                    groups = [(0, 1), (1, 1), (2, 2), (4, 4), (8, 4), (12, 4)] if qc == 0 \
                        else [(0, 4), (4, 4), (8, 4), (12, 4)]
                    gstarts = {a: n for a, n in groups}
                    for kt in range(NKT):
                        if kt in gstarts:
                            n = gstarts[kt]
                            g = slice(kt, kt + n)
                            if qc == 0:
                                # batched weight loads on the scalar HWDGE
                                # queue (a DMA descriptor costs ~700ns issue)
                                nc.scalar.dma_start(
                                    wq_sb[:, g, :], wq_d[g].rearrange("k p m -> p k m"))
                                nc.sync.dma_start(
                                    wk_sb[:, g, :], wk_d[g].rearrange("k p m -> p k m"))
                                nc.sync.dma_start(
                                    wv_sb[:, g, :], wv_d[g].rearrange("k p m -> p k m"))
                            xt4 = xpool.tile([128, 4, QC], BF16, tag="xt",
                                             name=f"x{qc}_{kt}")
                            nc.sync.dma_start(
                                xt4[:, 0:n, :], xT_d[g, :, qsl].rearrange("k p m -> p k m"))
                            xbase = kt
                            for _ in range(2):
                                if rope_q:
                                    rope_q.pop(0)()
                        if qc == 0 and kt == 13:
                            nc.scalar.dma_start(cos_sb[:], cos_d[:])
                            nc.scalar.dma_start(sin_sb[:], sin_d[:])
                        if qc == 0 and kt == 15:
                            nc.scalar.dma_start(tri_sb[:], tri_d[:])
                            nc.scalar.dma_start(wo_sb[:],
                                                wo_d.rearrange("h p m -> p h m"))
                        xt = xt4[:, kt - xbase, :]
                        st, sp = (kt == 0), (kt == NKT - 1)
                        for h in range(HQL):
                            nc.tensor.matmul(qps[h][:], wq_sb[:, kt, h * HD:(h + 1) * HD],
                                             xt, start=st, stop=sp)
                        for h in range(HKL):
                            nc.tensor.matmul(kps[h][:], wk_sb[:, kt, h * HD:(h + 1) * HD],
                                             xt, start=st, stop=sp)
                        for sb in range(4):
                            # two 256-col outputs share one PSUM bank: only the
                            # bank's first writer may clear has_written (start)
                            nc.tensor.matmul(vps[:, sb, :], xt4[:, kt - xbase, sb * 128:(sb + 1) * 128],
                                             wv_sb[:, kt, :],
                                             start=(st and sb % 2 == 0), stop=sp,
                                             skip_group_check=True)

                    if qc == 0:
                        # warm the rope tables on DVE so rope TTs carry a
                        # single cross-engine wait (the evac'd tile)
                        wmA = small.tile([1, 1], BF16, tag="warmA")
                        nc.vector.tensor_copy(wmA[:], cos_sb[0:1, 0:1])
                        wmB = small.tile([1, 1], BF16, tag="warmB")
                        nc.vector.tensor_copy(wmB[:], sin_sb[0:1, 0:1])
                        wmC = small.tile([1, 1], BF16, tag="warmC")
                        nc.vector.tensor_copy(wmC[:], tri_sb[0:1, 0:1])

                    # drain: evac copies split ACT/DVE in PE-consumption
                    # order so the 8 psum banks free fast; RoPE follows on DVE
                    evacs = []
                    srcs = [qps[0], qps[1], qps[2], qps[3], kps[0], kps[1]]
                    dsts = [qt_sb[0][qc], qt_sb[1][qc], qt_sb[2][qc],
                            qt_sb[3][qc], kt_sb[0][qc], kt_sb[1][qc]]
                    act_set = (0, 1, 5)     # ACT: q0, q1, k1 (then v)
                    for i, (ps, dst) in enumerate(zip(srcs, dsts)):
                        qsb = evpool.tile([128, QC], BF16, tag="evac",
                                          name=f"evac{qc}_{i}")
                        if i in act_set:
                            nc.scalar.copy(qsb[:], ps[:])
                        else:
                            nc.vector.tensor_copy(qsb[:], ps[:])
                        evacs.append((qsb, dst))
                    nc.scalar.copy(
                        v_sb[qc][:].rearrange("p a b -> p (a b)"),
                        vps[:].rearrange("p a b -> p (a b)"))

                    def rope_one(qsb, dst, qc=qc, i=0):
                        qsw = rtmp.tile([128, QC], BF16, tag="swap",
                                        name=f"swap{qc}_{i}")
                        nc.vector.tensor_copy(qsw[0:64, :], qsb[64:128, :])
                        nc.vector.tensor_copy(qsw[64:128, :], qsb[0:64, :])
                        a_t = rtmp.tile([128, QC], BF16, tag="ropeA")
                        nc.vector.tensor_tensor(a_t[:], qsb[:],
                                                cos_sb[:, qc * QC:(qc + 1) * QC],
                                                OP.mult)
                        b_t = rtmp.tile([128, QC], BF16, tag="ropeB")
                        nc.vector.tensor_tensor(b_t[:], qsw[:],
                                                sin_sb[:, qc * QC:(qc + 1) * QC],
                                                OP.mult)
                        nc.vector.tensor_tensor(dst[:], a_t[:], b_t[:], OP.add)

                    # defer this chunk's rope into the next chunk's kt
                    # loop (last chunk: into phase 2) so qc-boundary evacs
                    # never queue behind 8us of rope on the DVE
                    for i, (qsb, dst) in enumerate(evacs):
                        rope_q.append(
                            lambda qsb=qsb, dst=dst, qc=qc, i=i:
                            rope_one(qsb, dst, qc, i))

            # ---------- phase 2+3: attention with interleaved output proj ----
            with (
                tc.tile_pool(name="ps_s", bufs=2, space="PSUM") as ps_s,
                tc.tile_pool(name="ps_o", bufs=2, space="PSUM") as ps_o,
                tc.tile_pool(name="ps_z", bufs=1, space="PSUM") as ps_z,
                tc.tile_pool(name="ps_r", bufs=1, space="PSUM") as ps_r,
                tc.tile_pool(name="ps_f", bufs=2, space="PSUM") as ps_f,
            ):
                pending = [None]          # deferred rb-broadcast + normalize
                chunks = []               # deferred out-proj (sb, dcp) pairs
                qn = [0]                  # out-DMA queue alternator

                def emit_chunk(sb, dcp, tail=0):
                    o2 = outp.tile([128, 2, QC], F32, tag="osb",
                                   name=f"o{sb}_{dcp}")
                    for j in range(2):
                        dc = 2 * dcp + j
                        fps = ps_f.tile([128, QC], F32, tag="fps",
                                        name=f"fps{sb}_{dc}")
                        for h in range(HQL):
                            nc.tensor.matmul(
                                fps[:], ot_sb[h][sb // 4][:, (sb % 4) * 128:(sb % 4 + 1) * 128],
                                wo_sb[:, h, dc * QC:(dc + 1) * QC],
                                start=(h == 0), stop=(h == HQL - 1))
                        if j == 0 or tail:
                            nc.scalar.copy(o2[:, j, :], fps[:])
                        else:
                            nc.vector.tensor_copy(o2[:, j, :], fps[:])
                    eng = nc.sync if qn[0] % 2 == 0 else nc.scalar
                    qn[0] += 1
                    eng.dma_start(out_d[sb, :, dcp * 2 * QC:(dcp + 1) * 2 * QC],
                                  o2[:].rearrange("p a b -> p (a b)"))

                def flush_pending():
                    if pending[0] is not None:
                        pending[0]()
                        pending[0] = None

                # process q-chunks deepest-first-ish: the 4-kblock
                # qc=0 heads are chain-latency-bound, so run them LAST when
                # out-proj chunks exist to fill the PE bubbles
                for qc in (2, 3, 1, 0):
                    nkb = 4 * qc + 4
                    inj = 4 if nkb > 4 else 2   # chunk-injection stride
                    for h in range(HQL):
                        kv = h // 2
                        ops_t = ps_o.tile([128, QC], F32, tag="opv",
                                          name=f"opv{qc}_{h}")
                        zps_t = ps_z.tile([1, QC], F32, tag="zps",
                                          name=f"zps{qc}_{h}")
                        for kb in range(nkb):
                            off = (kb - 4 * qc) * 128 if kb >= 4 * qc else 0
                            sps = ps_s.tile([128, QC], F32, tag="sps",
                                            name=f"sps{qc}_{h}_{kb}")
                            nc.tensor.matmul(
                                sps[:, off:],
                                kt_sb[kv][kb // 4][:, (kb % 4) * 128:(kb % 4 + 1) * 128],
                                qt_sb[h][qc][:, off:],
                                start=True, stop=True)
                            e_t = epool.tile([128, QC], BF16, tag="etile",
                                             name=f"e{qc}_{h}_{kb}")
                            nc.scalar.activation(e_t[:, off:], sps[:, off:],
                                                 AF.Exp, scale=SCALE)
                            if kb >= 4 * qc:
                                nc.vector.tensor_tensor(
                                    e_t[:, off:off + 128], e_t[:, off:off + 128],
                                    tri_sb[:], OP.mult)
                            if kb == 1:
                                flush_pending()
                            st, sp = (kb == 0), (kb == nkb - 1)
                            nc.tensor.matmul(
                                ops_t[:, off:],
                                v_sb[kb // 4][:, kb % 4, kv * HD:(kv + 1) * HD],
                                e_t[:, off:], start=st, stop=sp,
                                skip_group_check=True)
                            nc.tensor.matmul(
                                zps_t[:, off:], ones_bf[:], e_t[:, off:],
                                start=st, stop=sp, skip_group_check=True)
                            if kb % inj == inj - 1 and chunks:
                                emit_chunk(*chunks.pop(0))

                        # deferred qc-3 rope: two tiles per head keeps
                        # the DVE queue shallow and finishes all 6 before
                        # qc-3 attention (processed second)
                        for _ in range(2):
                            if rope_q:
                                rope_q.pop(0)()

                        # reciprocal chain runs on DVE right away; the
                        # rb-broadcast matmul + normalize TT are deferred so
                        # the PE queue never waits on this chain
                        z_sb = small.tile([1, QC], F32, tag="zsb")
                        nc.vector.tensor_copy(z_sb[:], zps_t[:])
                        rz = small.tile([1, QC], F32, tag="rz")
                        nc.vector.reciprocal_approx_fast(rz[:], z_sb[:])
                        rz_h = small.tile([1, QC], FP16, tag="rzh")
                        nc.vector.tensor_copy(rz_h[:], rz[:])

                        def make_norm(h=h, qc=qc, ops_t=ops_t, rz_h=rz_h):
                            def norm():
                                rb_ps = ps_r.tile([128, QC], F32, tag="rbps",
                                                  name=f"rb{qc}_{h}")
                                nc.tensor.matmul(rb_ps[:], ones_h[:], rz_h[:],
                                                 start=True, stop=True)
                                rb_sb = small.tile([128, QC], F32, tag="rbsb",
                                                   name=f"rbs{qc}_{h}")
                                nc.vector.tensor_copy(rb_sb[:], rb_ps[:])
                                nc.vector.tensor_tensor(
                                    ot_sb[h][qc][:], ops_t[:], rb_sb[:],
                                    OP.mult)
                                if h == HQL - 1:
                                    for sb in range(4 * qc, 4 * qc + 4):
                                        for dcp in range(2):
                                            chunks.append((sb, dcp))
                            return norm

                        flush_pending()   # at most one outstanding
                        pending[0] = make_norm()

                flush_pending()
                while rope_q:
                    rope_q.pop(0)()
                ti = 0
                while chunks:
                    ti += 1
                    emit_chunk(*chunks.pop(0), tail=ti)

    nc.compile()
    return nc


_NC_CACHE = None


def _get_nc():
    global _NC_CACHE
    if _NC_CACHE is None:
        _NC_CACHE = _build_nc()
    return _NC_CACHE


def _rope_tables():
    inv = 1.0 / (ROPE_THETA ** (np.arange(0, HD, 2, dtype=np.float64) / HD))  # [64]
    t = np.arange(S, dtype=np.float64)
    ang = np.outer(inv, t)                      # [64, S]
    cos = np.cos(ang).astype(np.float32)
    sin = np.sin(ang).astype(np.float32)
    cos128 = np.concatenate([cos, cos], axis=0).astype(BF)  # [128, S]
    sinM = np.concatenate([-sin, sin], axis=0).astype(BF)
    return cos128, sinM


def prepare_inputs(x, wq, wk, wv, wo):
    """Build the 8 per-core input dicts from full inputs."""
    perm = np.concatenate([np.arange(0, HD, 2), np.arange(1, HD, 2)])
    cos128, sinM = _rope_tables()
    tri = np.greater_equal.outer(np.arange(128), np.arange(128)).T.astype(BF)

    x = np.asarray(x, np.float32)
    wq = np.asarray(wq, np.float32).reshape(HQ, HD, D)[:, perm, :]
    wk = np.asarray(wk, np.float32).reshape(HKV, HD, D)[:, perm, :]
    wv = np.asarray(wv, np.float32).reshape(HKV, HD, D)
    wo = np.asarray(wo, np.float32)              # [D, HQ*HD]

    in_maps = []
    for c in range(NCORES):
        b, hg = divmod(c, TP)
        qh = slice(hg * HQL, (hg + 1) * HQL)
        kh = slice(hg * HKL, (hg + 1) * HKL)
        xT = np.ascontiguousarray(x[b].T).astype(BF).reshape(NKT, 128, S)
        wq_t = np.ascontiguousarray(
            wq[qh].reshape(HQL * HD, D).T).astype(BF).reshape(NKT, 128, HQL * HD)
        wk_t = np.ascontiguousarray(
            wk[kh].reshape(HKL * HD, D).T).astype(BF).reshape(NKT, 128, HKL * HD)
        wv_t = np.ascontiguousarray(
            wv[kh].reshape(HKL * HD, D).T).astype(BF).reshape(NKT, 128, HKL * HD)
        wo_t = np.ascontiguousarray(
            wo[:, hg * HQL * HD:(hg + 1) * HQL * HD].T.reshape(HQL, HD, D)
        ).astype(BF)
        in_maps.append({
            "xT": xT, "wq_t": wq_t, "wk_t": wk_t, "wv_t": wv_t, "wo_t": wo_t,
            "cos128": cos128, "sinM": sinM, "tri": tri,
        })
    return in_maps


def _install_ntff_hook():
    """The agent image's antenv lacks axon_hooks; synthesize it so
    run_bass_kernel_spmd(trace=True) can capture NTFF profiles."""
    import sys as _sys
    import types, contextlib, ctypes

    if "antenv.axon_hooks" in _sys.modules:
        return
    so_path = "/opt/axon/libaxon_pjrt.so"
    lib = ctypes.CDLL(so_path)
    if not hasattr(lib, "axon_start_nrt_profile"):
        return
    lib.axon_start_nrt_profile.argtypes = [ctypes.POINTER(ctypes.c_int64),
                                           ctypes.c_size_t]
    lib.axon_start_nrt_profile.restype = ctypes.c_int64
    lib.axon_stop_nrt_profile.argtypes = [ctypes.c_char_p]
    lib.axon_stop_nrt_profile.restype = ctypes.c_int64

    @contextlib.contextmanager
    def _hook(output_dir, device_ids):
        import jax
        jax.devices()
        if device_ids:
            ids = (ctypes.c_int64 * len(device_ids))(*device_ids)
            rc = lib.axon_start_nrt_profile(ids, len(device_ids))
        else:
            rc = lib.axon_start_nrt_profile(None, 0)
        if rc != 0:
            raise RuntimeError(f"axon_start_nrt_profile rc={rc}")
        try:
            yield
        finally:
            n = lib.axon_stop_nrt_profile(str(output_dir).encode())
            print(f"ntff profile: {n} file(s) written to {output_dir}",
                  file=_sys.stderr)

    mod = types.ModuleType("antenv.axon_hooks")
    mod.get_axon_ntff_profile_hook = lambda: _hook
    mod.set_axon_ntff_profile_hook = lambda h: None
    _sys.modules["antenv.axon_hooks"] = mod
    try:
        import antenv
        antenv.axon_hooks = mod
    except ImportError:
        pass


def kernel(x, wq, wk, wv, wo, _trace=False, _trace_cores=None):
    in_maps = prepare_inputs(x, wq, wk, wv, wo)
    if _trace:
        _install_ntff_hook()
    nc = _get_nc()
    res = run_bass_kernel_spmd(
        nc, in_maps, core_ids=list(range(NCORES)),
        trace=_trace, trace_cores=_trace_cores)
    out = np.zeros((B, S, D), np.float32)
    for c in range(NCORES):
        b = c // TP
        out[b] += res.results[c]["out"].reshape(S, D)
    kernel.last_results = res
    return out


if __name__ == "__main__":
    rng = np.random.default_rng(0)
    x = rng.standard_normal((B, S, D), dtype=np.float32)
    sc = 1.0 / np.sqrt(D)
    wq = (rng.standard_normal((HQ * HD, D), dtype=np.float32) * sc)
    wk = (rng.standard_normal((HKV * HD, D), dtype=np.float32) * sc)
    wv = (rng.standard_normal((HKV * HD, D), dtype=np.float32) * sc)
    wo = (rng.standard_normal((D, HQ * HD), dtype=np.float32) * sc)
    out = kernel(x, wq, wk, wv, wo)
    print("ran", out.shape, out.dtype, float(np.abs(out).mean()))


# revision 17
# speedup vs baseline: 1.0195x; 1.0195x over previous
"""Causal self-attention (RoPE, GQA) on 8 Trainium2 NeuronCores.

Sharding: 2-way data-parallel over batch x 4-way tensor-parallel over heads.
Core c handles batch c//4 and head-group c%4 (4 q-heads, 2 kv-heads).
Each core computes its partial output projection (wo row-shard); the host
sums the 4 partials per batch (the "all-reduce" happens in the unshard step).

v3 layout/scheduling notes (empirically driven from the v1/v2 traces):
  - Everything on device is bf16 (fp32r moving operands stream at ~2x the
    bf16 rate on HW and fp32 LDWEIGHTS at 395ns can't hide under matmuls).
  - DMA descriptors cost ~700ns of issue time on the triggering engine, so
    weights/x are loaded in 4-kt batches, not per-tile.
  - qt/kt/ot/v live in per-(head, q-chunk) tiles: the Tile framework tracks
    dependencies at tile granularity, so a full-S tile would make phase-2
    readers wait for the LAST q-chunk's RoPE write (7.8us false stall).
  - Attention iterates per 128-wide k-block: S matmul -> exp -> (diag:
    128x128 triangle mask only) -> PV + Z accumulation, scores transposed
    [k, q] so exp output feeds PV directly.
  - Z via a ones-vector matmul; 1/Z broadcast with a K=1 fp16 matmul (fp32
    broadcast matmuls cost 4 cycles/row).  The broadcast + normalize for
    head h are DEFERRED past the next head's first k-block so the in-order
    PE queue never waits on the DVE reciprocal chain.
  - Output-projection pairs (8 matmuls -> one 512KB DMA, queues alternated)
    are injected into the attention loop of the NEXT q-chunk.
  - Phase-1 PSUM drain is split across ACT and DVE; RoPE runs SBUF-side in
    bf16 on the DVE.
"""

import sys
import numpy as np
import ml_dtypes

sys.path.insert(0, "/opt/trn_rl_repo")

import concourse.bass as bass
import concourse.bacc as bacc
import concourse.mybir as mybir
from concourse import tile
from concourse.bass_utils import run_bass_kernel_spmd

F32 = mybir.dt.float32
BF16 = mybir.dt.bfloat16
FP16 = mybir.dt.float16
AF = mybir.ActivationFunctionType
OP = mybir.AluOpType

B, S, D = 2, 2048, 2048
HQ, HKV, HD = 16, 8, 128
ROPE_THETA = 10000.0
NCORES, TP = 8, 4
HQL, HKL = HQ // TP, HKV // TP        # 4 q heads, 2 kv heads per core
NKT = D // 128                        # 16 contraction tiles
QC = 512                              # q-chunk width
NQC = S // QC                         # 4 q chunks
NSB = S // 128                        # 16 s-blocks
SCALE = 1.0 / float(np.sqrt(HD))
BF = ml_dtypes.bfloat16


def _build_nc():
    nc = bacc.Bacc("TRN2", target_bir_lowering=False)

    xT_d = nc.dram_tensor("xT", [NKT, 128, S], BF16, kind="ExternalInput")
    wq_d = nc.dram_tensor("wq_t", [NKT, 128, HQL * HD], BF16, kind="ExternalInput")
    wk_d = nc.dram_tensor("wk_t", [NKT, 128, HKL * HD], BF16, kind="ExternalInput")
    wv_d = nc.dram_tensor("wv_t", [NKT, 128, HKL * HD], BF16, kind="ExternalInput")
    wo_d = nc.dram_tensor("wo_t", [HQL, 128, D], BF16, kind="ExternalInput")
    cos_d = nc.dram_tensor("cos128", [128, S], BF16, kind="ExternalInput")
    sin_d = nc.dram_tensor("sinM", [128, S], BF16, kind="ExternalInput")
    tri_d = nc.dram_tensor("tri", [128, 128], BF16, kind="ExternalInput")
    out_d = nc.dram_tensor("out", [NSB, 128, D], F32, kind="ExternalOutput")

    with tile.TileContext(nc) as tc:
        with (
            tc.tile_pool(name="resident", bufs=1) as res,
            tc.tile_pool(name="xstream", bufs=3) as xpool,
            tc.tile_pool(name="ropetmp", bufs=2) as rtmp,
            tc.tile_pool(name="evpool", bufs=8) as evpool,
            tc.tile_pool(name="epool", bufs=4) as epool,
            tc.tile_pool(name="small", bufs=2) as small,
            tc.tile_pool(name="outp", bufs=2) as outp,
        ):
            # ---------- resident tiles ----------
            wq_sb = res.tile([128, NKT, HQL * HD], BF16)
            wk_sb = res.tile([128, NKT, HKL * HD], BF16)
            wv_sb = res.tile([128, NKT, HKL * HD], BF16)
            wo_sb = res.tile([128, HQL, D], BF16)
            cos_sb = res.tile([128, S], BF16)
            sin_sb = res.tile([128, S], BF16)
            tri_sb = res.tile([128, 128], BF16)

            ones_bf = res.tile([128, 1], BF16)
            nc.vector.memset(ones_bf[:], 1.0)
            ones_h = res.tile([1, 128], FP16)
            nc.vector.memset(ones_h[:], 1.0)

            # phase-1 outputs, one tile per (head, q-chunk) so phase-2
            # readers never pick up false deps on later chunks' writes
            qt_sb = [[res.tile([128, QC], BF16, tag=f"qt{h}_{c}", name=f"qt{h}_{c}")
                      for c in range(NQC)] for h in range(HQL)]
            kt_sb = [[res.tile([128, QC], BF16, tag=f"kt{h}_{c}", name=f"kt{h}_{c}")
                      for c in range(NQC)] for h in range(HKL)]
            v_sb = [res.tile([128, 4, HKL * HD], BF16, tag=f"v{c}", name=f"v{c}")
                    for c in range(NQC)]
            ot_sb = [[res.tile([128, QC], BF16, tag=f"ot{h}_{c}", name=f"ot{h}_{c}")
                      for c in range(NQC)] for h in range(HQL)]

            rope_q = []               # deferred qc-3 rope closures

            # ---------- phase 1: QKV projection + RoPE ----------
            with tc.tile_pool(name="ps1", bufs=1, space="PSUM") as ps1:
                for qc in range(NQC):
                    qsl = slice(qc * QC, (qc + 1) * QC)
                    qps = [ps1.tile([128, QC], F32, tag=f"qps{h}", name=f"qps{h}_{qc}") for h in range(HQL)]
                    kps = [ps1.tile([128, QC], F32, tag=f"kps{h}", name=f"kps{h}_{qc}") for h in range(HKL)]
                    vps = ps1.tile([128, 4, HKL * HD], F32, tag="vps")
                    # smaller first batches so the first matmuls start early
                    groups = [(0, 1), (1, 1), (2, 2), (4, 4), (8, 4), (12, 4)] if qc == 0 \
                        else [(0, 4), (4, 4), (8, 4), (12, 4)]
                    gstarts = {a: n for a, n in groups}
                    for kt in range(NKT):
                        if kt in gstarts:
                            n = gstarts[kt]
                            g = slice(kt, kt + n)
                            if qc == 0:
                                # batched weight loads on the scalar HWDGE
                                # queue (a DMA descriptor costs ~700ns issue)
                                nc.scalar.dma_start(
                                    wq_sb[:, g, :], wq_d[g].rearrange("k p m -> p k m"))
                                nc.scalar.dma_start(
                                    wk_sb[:, g, :], wk_d[g].rearrange("k p m -> p k m"))
                                nc.scalar.dma_start(
                                    wv_sb[:, g, :], wv_d[g].rearrange("k p m -> p k m"))
                            xt4 = xpool.tile([128, 4, QC], BF16, tag="xt",
                                             name=f"x{qc}_{kt}")
                            nc.sync.dma_start(
                                xt4[:, 0:n, :], xT_d[g, :, qsl].rearrange("k p m -> p k m"))
                            xbase = kt
                            for _ in range(2):
                                if rope_q:
                                    rope_q.pop(0)()
                        if qc == 0 and kt == 13:
                            nc.scalar.dma_start(cos_sb[:], cos_d[:])
                            nc.scalar.dma_start(sin_sb[:], sin_d[:])
                        if qc == 0 and kt == 15:
                            nc.scalar.dma_start(tri_sb[:], tri_d[:])
                            nc.scalar.dma_start(wo_sb[:],
                                                wo_d.rearrange("h p m -> p h m"))
                        xt = xt4[:, kt - xbase, :]
                        st, sp = (kt == 0), (kt == NKT - 1)
                        for h in range(HQL):
                            nc.tensor.matmul(qps[h][:], wq_sb[:, kt, h * HD:(h + 1) * HD],
                                             xt, start=st, stop=sp)
                        for h in range(HKL):
                            nc.tensor.matmul(kps[h][:], wk_sb[:, kt, h * HD:(h + 1) * HD],
                                             xt, start=st, stop=sp)
                        for sb in range(4):
                            # two 256-col outputs share one PSUM bank: only the
                            # bank's first writer may clear has_written (start)
                            nc.tensor.matmul(vps[:, sb, :], xt4[:, kt - xbase, sb * 128:(sb + 1) * 128],
                                             wv_sb[:, kt, :],
                                             start=(st and sb % 2 == 0), stop=sp,
                                             skip_group_check=True)

                    if qc == 0:
                        # warm the rope tables on DVE so rope TTs carry a
                        # single cross-engine wait (the evac'd tile)
                        wmA = small.tile([1, 1], BF16, tag="warmA")
                        nc.vector.tensor_copy(wmA[:], cos_sb[0:1, 0:1])
                        wmB = small.tile([1, 1], BF16, tag="warmB")
                        nc.vector.tensor_copy(wmB[:], sin_sb[0:1, 0:1])
                        wmC = small.tile([1, 1], BF16, tag="warmC")
                        nc.vector.tensor_copy(wmC[:], tri_sb[0:1, 0:1])

                    # drain: evac copies split ACT/DVE in PE-consumption
                    # order so the 8 psum banks free fast; RoPE follows on DVE
                    evacs = []
                    srcs = [qps[0], qps[1], qps[2], qps[3], kps[0], kps[1]]
                    dsts = [qt_sb[0][qc], qt_sb[1][qc], qt_sb[2][qc],
                            qt_sb[3][qc], kt_sb[0][qc], kt_sb[1][qc]]
                    act_set = (0, 1, 5)     # ACT: q0, q1, k1 (then v)
                    for i, (ps, dst) in enumerate(zip(srcs, dsts)):
                        qsb = evpool.tile([128, QC], BF16, tag="evac",
                                          name=f"evac{qc}_{i}")
                        if i in act_set:
                            nc.scalar.copy(qsb[:], ps[:])
                        else:
                            nc.vector.tensor_copy(qsb[:], ps[:])
                        evacs.append((qsb, dst))
                    nc.scalar.copy(
                        v_sb[qc][:].rearrange("p a b -> p (a b)"),
                        vps[:].rearrange("p a b -> p (a b)"))

                    def rope_one(qsb, dst, qc=qc, i=0):
                        qsw = rtmp.tile([128, QC], BF16, tag="swap",
                                        name=f"swap{qc}_{i}")
                        nc.vector.tensor_copy(qsw[0:64, :], qsb[64:128, :])
                        nc.vector.tensor_copy(qsw[64:128, :], qsb[0:64, :])
                        a_t = rtmp.tile([128, QC], BF16, tag="ropeA")
                        nc.vector.tensor_tensor(a_t[:], qsb[:],
                                                cos_sb[:, qc * QC:(qc + 1) * QC],
                                                OP.mult)
                        b_t = rtmp.tile([128, QC], BF16, tag="ropeB")
                        nc.vector.tensor_tensor(b_t[:], qsw[:],
                                                sin_sb[:, qc * QC:(qc + 1) * QC],
                                                OP.mult)
                        nc.vector.tensor_tensor(dst[:], a_t[:], b_t[:], OP.add)

                    # defer this chunk's rope into the next chunk's kt
                    # loop (last chunk: into phase 2) so qc-boundary evacs
                    # never queue behind 8us of rope on the DVE
                    for i, (qsb, dst) in enumerate(evacs):
                        rope_q.append(
                            lambda qsb=qsb, dst=dst, qc=qc, i=i:
                            rope_one(qsb, dst, qc, i))

            # ---------- phase 2+3: attention with interleaved output proj ----
            with (
                tc.tile_pool(name="ps_s", bufs=2, space="PSUM") as ps_s,
                tc.tile_pool(name="ps_o", bufs=2, space="PSUM") as ps_o,
                tc.tile_pool(name="ps_z", bufs=1, space="PSUM") as ps_z,
                tc.tile_pool(name="ps_r", bufs=1, space="PSUM") as ps_r,
                tc.tile_pool(name="ps_f", bufs=2, space="PSUM") as ps_f,
            ):
                pending = [None]          # deferred rb-broadcast + normalize
                chunks = []               # deferred out-proj (sb, dcp) pairs
                qn = [0]                  # out-DMA queue alternator

                def emit_chunk(sb, dcp, tail=0):
                    o2 = outp.tile([128, 2, QC], F32, tag="osb",
                                   name=f"o{sb}_{dcp}")
                    for j in range(2):
                        dc = 2 * dcp + j
                        fps = ps_f.tile([128, QC], F32, tag="fps",
                                        name=f"fps{sb}_{dc}")
                        for h in range(HQL):
                            nc.tensor.matmul(
                                fps[:], ot_sb[h][sb // 4][:, (sb % 4) * 128:(sb % 4 + 1) * 128],
                                wo_sb[:, h, dc * QC:(dc + 1) * QC],
                                start=(h == 0), stop=(h == HQL - 1))
                        if j == 0 or tail:
                            nc.scalar.copy(o2[:, j, :], fps[:])
                        else:
                            nc.vector.tensor_copy(o2[:, j, :], fps[:])
                    eng = nc.sync if qn[0] % 2 == 0 else nc.scalar
                    qn[0] += 1
                    eng.dma_start(out_d[sb, :, dcp * 2 * QC:(dcp + 1) * 2 * QC],
                                  o2[:].rearrange("p a b -> p (a b)"))

                def flush_pending():
                    if pending[0] is not None:
                        pending[0]()
                        pending[0] = None

                # process q-chunks deepest-first-ish: the 4-kblock
                # qc=0 heads are chain-latency-bound, so run them LAST when
                # out-proj chunks exist to fill the PE bubbles
                for qc in (2, 3, 1, 0):
                    nkb = 4 * qc + 4
                    inj = 4 if nkb > 4 else 2   # chunk-injection stride
                    for h in range(HQL):
                        kv = h // 2
                        ops_t = ps_o.tile([128, QC], F32, tag="opv",
                                          name=f"opv{qc}_{h}")
                        zps_t = ps_z.tile([1, QC], F32, tag="zps",
                                          name=f"zps{qc}_{h}")
                        for kb in range(nkb):
                            off = (kb - 4 * qc) * 128 if kb >= 4 * qc else 0
                            sps = ps_s.tile([128, QC], F32, tag="sps",
                                            name=f"sps{qc}_{h}_{kb}")
                            nc.tensor.matmul(
                                sps[:, off:],
                                kt_sb[kv][kb // 4][:, (kb % 4) * 128:(kb % 4 + 1) * 128],
                                qt_sb[h][qc][:, off:],
                                start=True, stop=True)
                            e_t = epool.tile([128, QC], BF16, tag="etile",
                                             name=f"e{qc}_{h}_{kb}")
                            nc.scalar.activation(e_t[:, off:], sps[:, off:],
                                                 AF.Exp, scale=SCALE)
                            if kb >= 4 * qc:
                                nc.vector.tensor_tensor(
                                    e_t[:, off:off + 128], e_t[:, off:off + 128],
                                    tri_sb[:], OP.mult)
                            if kb == 1:
                                flush_pending()
                            st, sp = (kb == 0), (kb == nkb - 1)
                            nc.tensor.matmul(
                                ops_t[:, off:],
                                v_sb[kb // 4][:, kb % 4, kv * HD:(kv + 1) * HD],
                                e_t[:, off:], start=st, stop=sp,
                                skip_group_check=True)
                            nc.tensor.matmul(
                                zps_t[:, off:], ones_bf[:], e_t[:, off:],
                                start=st, stop=sp, skip_group_check=True)
                            if kb % inj == inj - 1 and chunks:
                                emit_chunk(*chunks.pop(0))

                        # deferred qc-3 rope: two tiles per head keeps
                        # the DVE queue shallow and finishes all 6 before
                        # qc-3 attention (processed second)
                        for _ in range(2):
                            if rope_q:
                                rope_q.pop(0)()

                        # reciprocal chain runs on DVE right away; the
                        # rb-broadcast matmul + normalize TT are deferred so
                        # the PE queue never waits on this chain
                        z_sb = small.tile([1, QC], F32, tag="zsb")
                        nc.vector.tensor_copy(z_sb[:], zps_t[:])
                        rz = small.tile([1, QC], F32, tag="rz")
                        nc.vector.reciprocal_approx_fast(rz[:], z_sb[:])
                        rz_h = small.tile([1, QC], FP16, tag="rzh")
                        nc.vector.tensor_copy(rz_h[:], rz[:])

                        def make_norm(h=h, qc=qc, ops_t=ops_t, rz_h=rz_h):
                            def norm():
                                rb_ps = ps_r.tile([128, QC], F32, tag="rbps",
                                                  name=f"rb{qc}_{h}")
                                nc.tensor.matmul(rb_ps[:], ones_h[:], rz_h[:],
                                                 start=True, stop=True)
                                rb_sb = small.tile([128, QC], F32, tag="rbsb",
                                                   name=f"rbs{qc}_{h}")
                                nc.vector.tensor_copy(rb_sb[:], rb_ps[:])
                                nc.vector.tensor_tensor(
                                    ot_sb[h][qc][:], ops_t[:], rb_sb[:],
                                    OP.mult)
                                if h == HQL - 1:
                                    for sb in range(4 * qc, 4 * qc + 4):
                                        for dcp in range(2):
                                            chunks.append((sb, dcp))
                            return norm

                        flush_pending()   # at most one outstanding
                        pending[0] = make_norm()

                flush_pending()
                while rope_q:
                    rope_q.pop(0)()
                ti = 0
                while chunks:
                    ti += 1
                    emit_chunk(*chunks.pop(0), tail=ti)

    nc.compile()
    return nc


_NC_CACHE = None


def _get_nc():
    global _NC_CACHE
    if _NC_CACHE is None:
        _NC_CACHE = _build_nc()
    return _NC_CACHE


def _rope_tables():
    inv = 1.0 / (ROPE_THETA ** (np.arange(0, HD, 2, dtype=np.float64) / HD))  # [64]
    t = np.arange(S, dtype=np.float64)
    ang = np.outer(inv, t)                      # [64, S]
    cos = np.cos(ang).astype(np.float32)
    sin = np.sin(ang).astype(np.float32)
    cos128 = np.concatenate([cos, cos], axis=0).astype(BF)  # [128, S]
    sinM = np.concatenate([-sin, sin], axis=0).astype(BF)
    return cos128, sinM


def prepare_inputs(x, wq, wk, wv, wo):
    """Build the 8 per-core input dicts from full inputs."""
    perm = np.concatenate([np.arange(0, HD, 2), np.arange(1, HD, 2)])
    cos128, sinM = _rope_tables()
    tri = np.greater_equal.outer(np.arange(128), np.arange(128)).T.astype(BF)

    x = np.asarray(x, np.float32)
    wq = np.asarray(wq, np.float32).reshape(HQ, HD, D)[:, perm, :]
    wk = np.asarray(wk, np.float32).reshape(HKV, HD, D)[:, perm, :]
    wv = np.asarray(wv, np.float32).reshape(HKV, HD, D)
    wo = np.asarray(wo, np.float32)              # [D, HQ*HD]

    in_maps = []
    for c in range(NCORES):
        b, hg = divmod(c, TP)
        qh = slice(hg * HQL, (hg + 1) * HQL)
        kh = slice(hg * HKL, (hg + 1) * HKL)
        xT = np.ascontiguousarray(x[b].T).astype(BF).reshape(NKT, 128, S)
        wq_t = np.ascontiguousarray(
            wq[qh].reshape(HQL * HD, D).T).astype(BF).reshape(NKT, 128, HQL * HD)
        wk_t = np.ascontiguousarray(
            wk[kh].reshape(HKL * HD, D).T).astype(BF).reshape(NKT, 128, HKL * HD)
        wv_t = np.ascontiguousarray(
            wv[kh].reshape(HKL * HD, D).T).astype(BF).reshape(NKT, 128, HKL * HD)
        wo_t = np.ascontiguousarray(
            wo[:, hg * HQL * HD:(hg + 1) * HQL * HD].T.reshape(HQL, HD, D)
        ).astype(BF)
        in_maps.append({
            "xT": xT, "wq_t": wq_t, "wk_t": wk_t, "wv_t": wv_t, "wo_t": wo_t,
            "cos128": cos128, "sinM": sinM, "tri": tri,
        })
    return in_maps


def _install_ntff_hook():
    """The agent image's antenv lacks axon_hooks; synthesize it so
    run_bass_kernel_spmd(trace=True) can capture NTFF profiles."""
    import sys as _sys
    import types, contextlib, ctypes

    if "antenv.axon_hooks" in _sys.modules:
        return
    so_path = "/opt/axon/libaxon_pjrt.so"
    lib = ctypes.CDLL(so_path)
    if not hasattr(lib, "axon_start_nrt_profile"):
        return
    lib.axon_start_nrt_profile.argtypes = [ctypes.POINTER(ctypes.c_int64),
                                           ctypes.c_size_t]
    lib.axon_start_nrt_profile.restype = ctypes.c_int64
    lib.axon_stop_nrt_profile.argtypes = [ctypes.c_char_p]
    lib.axon_stop_nrt_profile.restype = ctypes.c_int64

    @contextlib.contextmanager
    def _hook(output_dir, device_ids):
        import jax
        jax.devices()
        if device_ids:
            ids = (ctypes.c_int64 * len(device_ids))(*device_ids)
            rc = lib.axon_start_nrt_profile(ids, len(device_ids))
        else:
            rc = lib.axon_start_nrt_profile(None, 0)
        if rc != 0:
            raise RuntimeError(f"axon_start_nrt_profile rc={rc}")
        try:
            yield
        finally:
            n = lib.axon_stop_nrt_profile(str(output_dir).encode())
            print(f"ntff profile: {n} file(s) written to {output_dir}",
                  file=_sys.stderr)

    mod = types.ModuleType("antenv.axon_hooks")
    mod.get_axon_ntff_profile_hook = lambda: _hook
    mod.set_axon_ntff_profile_hook = lambda h: None
    _sys.modules["antenv.axon_hooks"] = mod
    try:
        import antenv
        antenv.axon_hooks = mod
    except ImportError:
        pass


def kernel(x, wq, wk, wv, wo, _trace=False, _trace_cores=None):
    in_maps = prepare_inputs(x, wq, wk, wv, wo)
    if _trace:
        _install_ntff_hook()
    nc = _get_nc()
    res = run_bass_kernel_spmd(
        nc, in_maps, core_ids=list(range(NCORES)),
        trace=_trace, trace_cores=_trace_cores)
    out = np.zeros((B, S, D), np.float32)
    for c in range(NCORES):
        b = c // TP
        out[b] += res.results[c]["out"].reshape(S, D)
    kernel.last_results = res
    return out


if __name__ == "__main__":
    rng = np.random.default_rng(0)
    x = rng.standard_normal((B, S, D), dtype=np.float32)
    sc = 1.0 / np.sqrt(D)
    wq = (rng.standard_normal((HQ * HD, D), dtype=np.float32) * sc)
    wk = (rng.standard_normal((HKV * HD, D), dtype=np.float32) * sc)
    wv = (rng.standard_normal((HKV * HD, D), dtype=np.float32) * sc)
    wo = (rng.standard_normal((D, HQ * HD), dtype=np.float32) * sc)
    out = kernel(x, wq, wk, wv, wo)
    print("ran", out.shape, out.dtype, float(np.abs(out).mean()))


# revision 18
# speedup vs baseline: 1.0223x; 1.0027x over previous
"""Causal self-attention (RoPE, GQA) on 8 Trainium2 NeuronCores.

Sharding: 2-way data-parallel over batch x 4-way tensor-parallel over heads.
Core c handles batch c//4 and head-group c%4 (4 q-heads, 2 kv-heads).
Each core computes its partial output projection (wo row-shard); the host
sums the 4 partials per batch (the "all-reduce" happens in the unshard step).

v3 layout/scheduling notes (empirically driven from the v1/v2 traces):
  - Everything on device is bf16 (fp32r moving operands stream at ~2x the
    bf16 rate on HW and fp32 LDWEIGHTS at 395ns can't hide under matmuls).
  - DMA descriptors cost ~700ns of issue time on the triggering engine, so
    weights/x are loaded in 4-kt batches, not per-tile.
  - qt/kt/ot/v live in per-(head, q-chunk) tiles: the Tile framework tracks
    dependencies at tile granularity, so a full-S tile would make phase-2
    readers wait for the LAST q-chunk's RoPE write (7.8us false stall).
  - Attention iterates per 128-wide k-block: S matmul -> exp -> (diag:
    128x128 triangle mask only) -> PV + Z accumulation, scores transposed
    [k, q] so exp output feeds PV directly.
  - Z via a ones-vector matmul; 1/Z broadcast with a K=1 fp16 matmul (fp32
    broadcast matmuls cost 4 cycles/row).  The broadcast + normalize for
    head h are DEFERRED past the next head's first k-block so the in-order
    PE queue never waits on the DVE reciprocal chain.
  - Output-projection pairs (8 matmuls -> one 512KB DMA, queues alternated)
    are injected into the attention loop of the NEXT q-chunk.
  - Phase-1 PSUM drain is split across ACT and DVE; RoPE runs SBUF-side in
    bf16 on the DVE.
"""

import sys
import numpy as np
import ml_dtypes

sys.path.insert(0, "/opt/trn_rl_repo")

import concourse.bass as bass
import concourse.bacc as bacc
import concourse.mybir as mybir
from concourse import tile
from concourse.bass_utils import run_bass_kernel_spmd

F32 = mybir.dt.float32
BF16 = mybir.dt.bfloat16
FP16 = mybir.dt.float16
AF = mybir.ActivationFunctionType
OP = mybir.AluOpType

B, S, D = 2, 2048, 2048
HQ, HKV, HD = 16, 8, 128
ROPE_THETA = 10000.0
NCORES, TP = 8, 4
HQL, HKL = HQ // TP, HKV // TP        # 4 q heads, 2 kv heads per core
NKT = D // 128                        # 16 contraction tiles
QC = 512                              # q-chunk width
NQC = S // QC                         # 4 q chunks
NSB = S // 128                        # 16 s-blocks
SCALE = 1.0 / float(np.sqrt(HD))
BF = ml_dtypes.bfloat16


def _build_nc():
    nc = bacc.Bacc("TRN2", target_bir_lowering=False)

    xT_d = nc.dram_tensor("xT", [NKT, 128, S], BF16, kind="ExternalInput")
    wq_d = nc.dram_tensor("wq_t", [NKT, 128, HQL * HD], BF16, kind="ExternalInput")
    wk_d = nc.dram_tensor("wk_t", [NKT, 128, HKL * HD], BF16, kind="ExternalInput")
    wv_d = nc.dram_tensor("wv_t", [NKT, 128, HKL * HD], BF16, kind="ExternalInput")
    wo_d = nc.dram_tensor("wo_t", [HQL, 128, D], BF16, kind="ExternalInput")
    cos_d = nc.dram_tensor("cos128", [128, S], BF16, kind="ExternalInput")
    sin_d = nc.dram_tensor("sinM", [128, S], BF16, kind="ExternalInput")
    tri_d = nc.dram_tensor("tri", [128, 128], BF16, kind="ExternalInput")
    out_d = nc.dram_tensor("out", [NSB, 128, D], F32, kind="ExternalOutput")

    with tile.TileContext(nc) as tc:
        with (
            tc.tile_pool(name="resident", bufs=1) as res,
            tc.tile_pool(name="xstream", bufs=3) as xpool,
            tc.tile_pool(name="ropetmp", bufs=2) as rtmp,
            tc.tile_pool(name="evpool", bufs=8) as evpool,
            tc.tile_pool(name="epool", bufs=6) as epool,
            tc.tile_pool(name="small", bufs=2) as small,
            tc.tile_pool(name="outp", bufs=4) as outp,
        ):
            # ---------- resident tiles ----------
            wq_sb = res.tile([128, NKT, HQL * HD], BF16)
            wk_sb = res.tile([128, NKT, HKL * HD], BF16)
            wv_sb = res.tile([128, NKT, HKL * HD], BF16)
            wo_sb = res.tile([128, HQL, D], BF16)
            cos_sb = res.tile([128, S], BF16)
            sin_sb = res.tile([128, S], BF16)
            tri_sb = res.tile([128, 128], BF16)

            ones_bf = res.tile([128, 1], BF16)
            nc.vector.memset(ones_bf[:], 1.0)
            ones_h = res.tile([1, 128], FP16)
            nc.vector.memset(ones_h[:], 1.0)

            # phase-1 outputs, one tile per (head, q-chunk) so phase-2
            # readers never pick up false deps on later chunks' writes
            qt_sb = [[res.tile([128, QC], BF16, tag=f"qt{h}_{c}", name=f"qt{h}_{c}")
                      for c in range(NQC)] for h in range(HQL)]
            kt_sb = [[res.tile([128, QC], BF16, tag=f"kt{h}_{c}", name=f"kt{h}_{c}")
                      for c in range(NQC)] for h in range(HKL)]
            v_sb = [res.tile([128, 4, HKL * HD], BF16, tag=f"v{c}", name=f"v{c}")
                    for c in range(NQC)]
            ot_sb = [[res.tile([128, QC], BF16, tag=f"ot{h}_{c}", name=f"ot{h}_{c}")
                      for c in range(NQC)] for h in range(HQL)]

            rope_q = []               # deferred qc-3 rope closures

            # ---------- phase 1: QKV projection + RoPE ----------
            with tc.tile_pool(name="ps1", bufs=1, space="PSUM") as ps1:
                for qc in range(NQC):
                    qsl = slice(qc * QC, (qc + 1) * QC)
                    qps = [ps1.tile([128, QC], F32, tag=f"qps{h}", name=f"qps{h}_{qc}") for h in range(HQL)]
                    kps = [ps1.tile([128, QC], F32, tag=f"kps{h}", name=f"kps{h}_{qc}") for h in range(HKL)]
                    vps = ps1.tile([128, 4, HKL * HD], F32, tag="vps")
                    # smaller first batches so the first matmuls start early
                    groups = [(0, 1), (1, 1), (2, 2), (4, 4), (8, 4), (12, 4)] if qc == 0 \
                        else [(0, 4), (4, 4), (8, 4), (12, 4)]
                    gstarts = {a: n for a, n in groups}
                    for kt in range(NKT):
                        if kt in gstarts:
                            n = gstarts[kt]
                            g = slice(kt, kt + n)
                            if qc == 0:
                                # batched weight loads on the scalar HWDGE
                                # queue (a DMA descriptor costs ~700ns issue)
                                nc.scalar.dma_start(
                                    wq_sb[:, g, :], wq_d[g].rearrange("k p m -> p k m"))
                                nc.scalar.dma_start(
                                    wk_sb[:, g, :], wk_d[g].rearrange("k p m -> p k m"))
                                nc.scalar.dma_start(
                                    wv_sb[:, g, :], wv_d[g].rearrange("k p m -> p k m"))
                            xt4 = xpool.tile([128, 4, QC], BF16, tag="xt",
                                             name=f"x{qc}_{kt}")
                            nc.sync.dma_start(
                                xt4[:, 0:n, :], xT_d[g, :, qsl].rearrange("k p m -> p k m"))
                            xbase = kt
                            for _ in range(2):
                                if rope_q:
                                    rope_q.pop(0)()
                        if qc == 0 and kt == 13:
                            nc.scalar.dma_start(cos_sb[:], cos_d[:])
                            nc.scalar.dma_start(sin_sb[:], sin_d[:])
                        if qc == 0 and kt == 15:
                            nc.scalar.dma_start(tri_sb[:], tri_d[:])
                        if qc == 1 and kt == 12:
                            # wo is first read ~30us into phase 2; loading it
                            # here keeps it out of the congested startup window
                            nc.scalar.dma_start(wo_sb[:],
                                                wo_d.rearrange("h p m -> p h m"))
                        xt = xt4[:, kt - xbase, :]
                        st, sp = (kt == 0), (kt == NKT - 1)
                        for h in range(HQL):
                            nc.tensor.matmul(qps[h][:], wq_sb[:, kt, h * HD:(h + 1) * HD],
                                             xt, start=st, stop=sp)
                        for h in range(HKL):
                            nc.tensor.matmul(kps[h][:], wk_sb[:, kt, h * HD:(h + 1) * HD],
                                             xt, start=st, stop=sp)
                        for sb in range(4):
                            # two 256-col outputs share one PSUM bank: only the
                            # bank's first writer may clear has_written (start)
                            nc.tensor.matmul(vps[:, sb, :], xt4[:, kt - xbase, sb * 128:(sb + 1) * 128],
                                             wv_sb[:, kt, :],
                                             start=(st and sb % 2 == 0), stop=sp,
                                             skip_group_check=True)

                    if qc == 0:
                        # warm the rope tables on DVE so rope TTs carry a
                        # single cross-engine wait (the evac'd tile)
                        wmA = small.tile([1, 1], BF16, tag="warmA")
                        nc.vector.tensor_copy(wmA[:], cos_sb[0:1, 0:1])
                        wmB = small.tile([1, 1], BF16, tag="warmB")
                        nc.vector.tensor_copy(wmB[:], sin_sb[0:1, 0:1])
                        wmC = small.tile([1, 1], BF16, tag="warmC")
                        nc.vector.tensor_copy(wmC[:], tri_sb[0:1, 0:1])

                    # drain: evac copies split ACT/DVE in PE-consumption
                    # order so the 8 psum banks free fast; RoPE follows on DVE
                    evacs = []
                    srcs = [qps[0], qps[1], qps[2], qps[3], kps[0], kps[1]]
                    dsts = [qt_sb[0][qc], qt_sb[1][qc], qt_sb[2][qc],
                            qt_sb[3][qc], kt_sb[0][qc], kt_sb[1][qc]]
                    act_set = (0, 1, 5)     # ACT: q0, q1, k1 (then v)
                    for i, (ps, dst) in enumerate(zip(srcs, dsts)):
                        qsb = evpool.tile([128, QC], BF16, tag="evac",
                                          name=f"evac{qc}_{i}")
                        if i in act_set:
                            nc.scalar.copy(qsb[:], ps[:])
                        else:
                            nc.vector.tensor_copy(qsb[:], ps[:])
                        evacs.append((qsb, dst))
                    nc.scalar.copy(
                        v_sb[qc][:].rearrange("p a b -> p (a b)"),
                        vps[:].rearrange("p a b -> p (a b)"))

                    def rope_one(qsb, dst, qc=qc, i=0):
                        qsw = rtmp.tile([128, QC], BF16, tag="swap",
                                        name=f"swap{qc}_{i}")
                        nc.vector.tensor_copy(qsw[0:64, :], qsb[64:128, :])
                        nc.vector.tensor_copy(qsw[64:128, :], qsb[0:64, :])
                        a_t = rtmp.tile([128, QC], BF16, tag="ropeA")
                        nc.vector.tensor_tensor(a_t[:], qsb[:],
                                                cos_sb[:, qc * QC:(qc + 1) * QC],
                                                OP.mult)
                        b_t = rtmp.tile([128, QC], BF16, tag="ropeB")
                        nc.vector.tensor_tensor(b_t[:], qsw[:],
                                                sin_sb[:, qc * QC:(qc + 1) * QC],
                                                OP.mult)
                        nc.vector.tensor_tensor(dst[:], a_t[:], b_t[:], OP.add)

                    # defer this chunk's rope into the next chunk's kt
                    # loop (last chunk: into phase 2) so qc-boundary evacs
                    # never queue behind 8us of rope on the DVE
                    for i, (qsb, dst) in enumerate(evacs):
                        rope_q.append(
                            lambda qsb=qsb, dst=dst, qc=qc, i=i:
                            rope_one(qsb, dst, qc, i))

            # ---------- phase 2+3: attention with interleaved output proj ----
            with (
                tc.tile_pool(name="ps_s", bufs=2, space="PSUM") as ps_s,
                tc.tile_pool(name="ps_o", bufs=2, space="PSUM") as ps_o,
                tc.tile_pool(name="ps_z", bufs=1, space="PSUM") as ps_z,
                tc.tile_pool(name="ps_r", bufs=1, space="PSUM") as ps_r,
                tc.tile_pool(name="ps_f", bufs=2, space="PSUM") as ps_f,
            ):
                pending = [None]          # deferred rb-broadcast + normalize
                chunks = []               # deferred out-proj (sb, dcp) pairs
                qn = [0]                  # out-DMA queue alternator

                def emit_chunk(sb, dcp, tail=0):
                    o2 = outp.tile([128, 2, QC], F32, tag="osb",
                                   name=f"o{sb}_{dcp}")
                    for j in range(2):
                        dc = 2 * dcp + j
                        fps = ps_f.tile([128, QC], F32, tag="fps",
                                        name=f"fps{sb}_{dc}")
                        for h in range(HQL):
                            nc.tensor.matmul(
                                fps[:], ot_sb[h][sb // 4][:, (sb % 4) * 128:(sb % 4 + 1) * 128],
                                wo_sb[:, h, dc * QC:(dc + 1) * QC],
                                start=(h == 0), stop=(h == HQL - 1))
                        if j == 0 or tail:
                            nc.scalar.copy(o2[:, j, :], fps[:])
                        else:
                            nc.vector.tensor_copy(o2[:, j, :], fps[:])
                    eng = nc.sync if qn[0] % 2 == 0 else nc.scalar
                    qn[0] += 1
                    eng.dma_start(out_d[sb, :, dcp * 2 * QC:(dcp + 1) * 2 * QC],
                                  o2[:].rearrange("p a b -> p (a b)"))

                def flush_pending():
                    if pending[0] is not None:
                        pending[0]()
                        pending[0] = None

                # process q-chunks deepest-first-ish: the 4-kblock
                # qc=0 heads are chain-latency-bound, so run them LAST when
                # out-proj chunks exist to fill the PE bubbles
                for qc in (2, 3, 1, 0):
                    nkb = 4 * qc + 4
                    inj = 4 if nkb > 4 else 2   # chunk-injection stride
                    for h in range(HQL):
                        kv = h // 2
                        ops_t = ps_o.tile([128, QC], F32, tag="opv",
                                          name=f"opv{qc}_{h}")
                        zps_t = ps_z.tile([1, QC], F32, tag="zps",
                                          name=f"zps{qc}_{h}")
                        for kb in range(nkb):
                            off = (kb - 4 * qc) * 128 if kb >= 4 * qc else 0
                            sps = ps_s.tile([128, QC], F32, tag="sps",
                                            name=f"sps{qc}_{h}_{kb}")
                            nc.tensor.matmul(
                                sps[:, off:],
                                kt_sb[kv][kb // 4][:, (kb % 4) * 128:(kb % 4 + 1) * 128],
                                qt_sb[h][qc][:, off:],
                                start=True, stop=True)
                            e_t = epool.tile([128, QC], BF16, tag="etile",
                                             name=f"e{qc}_{h}_{kb}")
                            nc.scalar.activation(e_t[:, off:], sps[:, off:],
                                                 AF.Exp, scale=SCALE)
                            if kb >= 4 * qc:
                                nc.vector.tensor_tensor(
                                    e_t[:, off:off + 128], e_t[:, off:off + 128],
                                    tri_sb[:], OP.mult)
                            if kb == 1:
                                flush_pending()
                            st, sp = (kb == 0), (kb == nkb - 1)
                            nc.tensor.matmul(
                                ops_t[:, off:],
                                v_sb[kb // 4][:, kb % 4, kv * HD:(kv + 1) * HD],
                                e_t[:, off:], start=st, stop=sp,
                                skip_group_check=True)
                            nc.tensor.matmul(
                                zps_t[:, off:], ones_bf[:], e_t[:, off:],
                                start=st, stop=sp, skip_group_check=True)
                            if kb % inj == inj - 1 and chunks:
                                emit_chunk(*chunks.pop(0))

                        # deferred qc-3 rope: two tiles per head keeps
                        # the DVE queue shallow and finishes all 6 before
                        # qc-3 attention (processed second)
                        for _ in range(2):
                            if rope_q:
                                rope_q.pop(0)()

                        # reciprocal chain runs on DVE right away; the
                        # rb-broadcast matmul + normalize TT are deferred so
                        # the PE queue never waits on this chain
                        z_sb = small.tile([1, QC], F32, tag="zsb")
                        nc.vector.tensor_copy(z_sb[:], zps_t[:])
                        rz = small.tile([1, QC], F32, tag="rz")
                        nc.vector.reciprocal_approx_fast(rz[:], z_sb[:])
                        rz_h = small.tile([1, QC], FP16, tag="rzh")
                        nc.vector.tensor_copy(rz_h[:], rz[:])

                        def make_norm(h=h, qc=qc, ops_t=ops_t, rz_h=rz_h):
                            def norm():
                                rb_ps = ps_r.tile([128, QC], F32, tag="rbps",
                                                  name=f"rb{qc}_{h}")
                                nc.tensor.matmul(rb_ps[:], ones_h[:], rz_h[:],
                                                 start=True, stop=True)
                                rb_sb = small.tile([128, QC], F32, tag="rbsb",
                                                   name=f"rbs{qc}_{h}")
                                nc.vector.tensor_copy(rb_sb[:], rb_ps[:])
                                nc.vector.tensor_tensor(
                                    ot_sb[h][qc][:], ops_t[:], rb_sb[:],
                                    OP.mult)
                                if h == HQL - 1:
                                    for sb in range(4 * qc, 4 * qc + 4):
                                        for dcp in range(2):
                                            chunks.append((sb, dcp))
                            return norm

                        flush_pending()   # at most one outstanding
                        pending[0] = make_norm()

                flush_pending()
                while rope_q:
                    rope_q.pop(0)()
                ti = 0
                while chunks:
                    ti += 1
                    emit_chunk(*chunks.pop(0), tail=ti)

    nc.compile()
    return nc


_NC_CACHE = None


def _get_nc():
    global _NC_CACHE
    if _NC_CACHE is None:
        _NC_CACHE = _build_nc()
    return _NC_CACHE


def _rope_tables():
    inv = 1.0 / (ROPE_THETA ** (np.arange(0, HD, 2, dtype=np.float64) / HD))  # [64]
    t = np.arange(S, dtype=np.float64)
    ang = np.outer(inv, t)                      # [64, S]
    cos = np.cos(ang).astype(np.float32)
    sin = np.sin(ang).astype(np.float32)
    cos128 = np.concatenate([cos, cos], axis=0).astype(BF)  # [128, S]
    sinM = np.concatenate([-sin, sin], axis=0).astype(BF)
    return cos128, sinM


def prepare_inputs(x, wq, wk, wv, wo):
    """Build the 8 per-core input dicts from full inputs."""
    perm = np.concatenate([np.arange(0, HD, 2), np.arange(1, HD, 2)])
    cos128, sinM = _rope_tables()
    tri = np.greater_equal.outer(np.arange(128), np.arange(128)).T.astype(BF)

    x = np.asarray(x, np.float32)
    wq = np.asarray(wq, np.float32).reshape(HQ, HD, D)[:, perm, :]
    wk = np.asarray(wk, np.float32).reshape(HKV, HD, D)[:, perm, :]
    wv = np.asarray(wv, np.float32).reshape(HKV, HD, D)
    wo = np.asarray(wo, np.float32)              # [D, HQ*HD]

    in_maps = []
    for c in range(NCORES):
        b, hg = divmod(c, TP)
        qh = slice(hg * HQL, (hg + 1) * HQL)
        kh = slice(hg * HKL, (hg + 1) * HKL)
        xT = np.ascontiguousarray(x[b].T).astype(BF).reshape(NKT, 128, S)
        wq_t = np.ascontiguousarray(
            wq[qh].reshape(HQL * HD, D).T).astype(BF).reshape(NKT, 128, HQL * HD)
        wk_t = np.ascontiguousarray(
            wk[kh].reshape(HKL * HD, D).T).astype(BF).reshape(NKT, 128, HKL * HD)
        wv_t = np.ascontiguousarray(
            wv[kh].reshape(HKL * HD, D).T).astype(BF).reshape(NKT, 128, HKL * HD)
        wo_t = np.ascontiguousarray(
            wo[:, hg * HQL * HD:(hg + 1) * HQL * HD].T.reshape(HQL, HD, D)
        ).astype(BF)
        in_maps.append({
            "xT": xT, "wq_t": wq_t, "wk_t": wk_t, "wv_t": wv_t, "wo_t": wo_t,
            "cos128": cos128, "sinM": sinM, "tri": tri,
        })
    return in_maps


def _install_ntff_hook():
    """The agent image's antenv lacks axon_hooks; synthesize it so
    run_bass_kernel_spmd(trace=True) can capture NTFF profiles."""
    import sys as _sys
    import types, contextlib, ctypes

    if "antenv.axon_hooks" in _sys.modules:
        return
    so_path = "/opt/axon/libaxon_pjrt.so"
    lib = ctypes.CDLL(so_path)
    if not hasattr(lib, "axon_start_nrt_profile"):
        return
    lib.axon_start_nrt_profile.argtypes = [ctypes.POINTER(ctypes.c_int64),
                                           ctypes.c_size_t]
    lib.axon_start_nrt_profile.restype = ctypes.c_int64
    lib.axon_stop_nrt_profile.argtypes = [ctypes.c_char_p]
    lib.axon_stop_nrt_profile.restype = ctypes.c_int64

    @contextlib.contextmanager
    def _hook(output_dir, device_ids):
        import jax
        jax.devices()
        if device_ids:
            ids = (ctypes.c_int64 * len(device_ids))(*device_ids)
            rc = lib.axon_start_nrt_profile(ids, len(device_ids))
        else:
            rc = lib.axon_start_nrt_profile(None, 0)
        if rc != 0:
            raise RuntimeError(f"axon_start_nrt_profile rc={rc}")
        try:
            yield
        finally:
            n = lib.axon_stop_nrt_profile(str(output_dir).encode())
            print(f"ntff profile: {n} file(s) written to {output_dir}",
                  file=_sys.stderr)

    mod = types.ModuleType("antenv.axon_hooks")
    mod.get_axon_ntff_profile_hook = lambda: _hook
    mod.set_axon_ntff_profile_hook = lambda h: None
    _sys.modules["antenv.axon_hooks"] = mod
    try:
        import antenv
        antenv.axon_hooks = mod
    except ImportError:
        pass


def kernel(x, wq, wk, wv, wo, _trace=False, _trace_cores=None):
    in_maps = prepare_inputs(x, wq, wk, wv, wo)
    if _trace:
        _install_ntff_hook()
    nc = _get_nc()
    res = run_bass_kernel_spmd(
        nc, in_maps, core_ids=list(range(NCORES)),
        trace=_trace, trace_cores=_trace_cores)
    out = np.zeros((B, S, D), np.float32)
    for c in range(NCORES):
        b = c // TP
        out[b] += res.results[c]["out"].reshape(S, D)
    kernel.last_results = res
    return out


if __name__ == "__main__":
    rng = np.random.default_rng(0)
    x = rng.standard_normal((B, S, D), dtype=np.float32)
    sc = 1.0 / np.sqrt(D)
    wq = (rng.standard_normal((HQ * HD, D), dtype=np.float32) * sc)
    wk = (rng.standard_normal((HKV * HD, D), dtype=np.float32) * sc)
    wv = (rng.standard_normal((HKV * HD, D), dtype=np.float32) * sc)
    wo = (rng.standard_normal((D, HQ * HD), dtype=np.float32) * sc)
    out = kernel(x, wq, wk, wv, wo)
    print("ran", out.shape, out.dtype, float(np.abs(out).mean()))


# revision 20
# speedup vs baseline: 1.0255x; 1.0031x over previous
"""Causal self-attention (RoPE, GQA) on 8 Trainium2 NeuronCores.

Sharding: 2-way data-parallel over batch x 4-way tensor-parallel over heads.
Core c handles batch c//4 and head-group c%4 (4 q-heads, 2 kv-heads).
Each core computes its partial output projection (wo row-shard); the host
sums the 4 partials per batch (the "all-reduce" happens in the unshard step).

v3 layout/scheduling notes (empirically driven from the v1/v2 traces):
  - Everything on device is bf16 (fp32r moving operands stream at ~2x the
    bf16 rate on HW and fp32 LDWEIGHTS at 395ns can't hide under matmuls).
  - DMA descriptors cost ~700ns of issue time on the triggering engine, so
    weights/x are loaded in 4-kt batches, not per-tile.
  - qt/kt/ot/v live in per-(head, q-chunk) tiles: the Tile framework tracks
    dependencies at tile granularity, so a full-S tile would make phase-2
    readers wait for the LAST q-chunk's RoPE write (7.8us false stall).
  - Attention iterates per 128-wide k-block: S matmul -> exp -> (diag:
    128x128 triangle mask only) -> PV + Z accumulation, scores transposed
    [k, q] so exp output feeds PV directly.
  - Z via a ones-vector matmul; 1/Z broadcast with a K=1 fp16 matmul (fp32
    broadcast matmuls cost 4 cycles/row).  The broadcast + normalize for
    head h are DEFERRED past the next head's first k-block so the in-order
    PE queue never waits on the DVE reciprocal chain.
  - Output-projection pairs (8 matmuls -> one 512KB DMA, queues alternated)
    are injected into the attention loop of the NEXT q-chunk.
  - Phase-1 PSUM drain is split across ACT and DVE; RoPE runs SBUF-side in
    bf16 on the DVE.
"""

import sys
import numpy as np
import ml_dtypes

sys.path.insert(0, "/opt/trn_rl_repo")

import concourse.bass as bass
import concourse.bacc as bacc
import concourse.mybir as mybir
from concourse import tile
from concourse.bass_utils import run_bass_kernel_spmd

F32 = mybir.dt.float32
BF16 = mybir.dt.bfloat16
FP16 = mybir.dt.float16
AF = mybir.ActivationFunctionType
OP = mybir.AluOpType

B, S, D = 2, 2048, 2048
HQ, HKV, HD = 16, 8, 128
ROPE_THETA = 10000.0
NCORES, TP = 8, 4
HQL, HKL = HQ // TP, HKV // TP        # 4 q heads, 2 kv heads per core
NKT = D // 128                        # 16 contraction tiles
QC = 512                              # q-chunk width
NQC = S // QC                         # 4 q chunks
NSB = S // 128                        # 16 s-blocks
SCALE = 1.0 / float(np.sqrt(HD))
BF = ml_dtypes.bfloat16


def _build_nc():
    nc = bacc.Bacc("TRN2", target_bir_lowering=False)

    xT_d = nc.dram_tensor("xT", [NKT, 128, S], BF16, kind="ExternalInput")
    wq_d = nc.dram_tensor("wq_t", [NKT, 128, HQL * HD], BF16, kind="ExternalInput")
    wk_d = nc.dram_tensor("wk_t", [NKT, 128, HKL * HD], BF16, kind="ExternalInput")
    wv_d = nc.dram_tensor("wv_t", [NKT, 128, HKL * HD], BF16, kind="ExternalInput")
    wo_d = nc.dram_tensor("wo_t", [HQL, 128, D], BF16, kind="ExternalInput")
    cos_d = nc.dram_tensor("cos128", [128, S], BF16, kind="ExternalInput")
    sin_d = nc.dram_tensor("sinM", [128, S], BF16, kind="ExternalInput")
    tri_d = nc.dram_tensor("tri", [128, 128], BF16, kind="ExternalInput")
    # bf16 output halves the out-DMA traffic (the kernel tail is
    # DMA-drain-bound); the host converts back to fp32 in the unshard
    out_d = nc.dram_tensor("out", [NSB, 128, D], BF16, kind="ExternalOutput")

    with tile.TileContext(nc) as tc:
        with (
            tc.tile_pool(name="resident", bufs=1) as res,
            tc.tile_pool(name="xstream", bufs=3) as xpool,
            tc.tile_pool(name="ropetmp", bufs=2) as rtmp,
            tc.tile_pool(name="evpool", bufs=8) as evpool,
            tc.tile_pool(name="epool", bufs=6) as epool,
            tc.tile_pool(name="small", bufs=2) as small,
            tc.tile_pool(name="outp", bufs=4) as outp,
        ):
            # ---------- resident tiles ----------
            wq_sb = res.tile([128, NKT, HQL * HD], BF16)
            wk_sb = res.tile([128, NKT, HKL * HD], BF16)
            wv_sb = res.tile([128, NKT, HKL * HD], BF16)
            wo_sb = res.tile([128, HQL, D], BF16)
            cos_sb = res.tile([128, S], BF16)
            sin_sb = res.tile([128, S], BF16)
            tri_sb = res.tile([128, 128], BF16)

            ones_bf = res.tile([128, 1], BF16)
            nc.vector.memset(ones_bf[:], 1.0)
            ones_h = res.tile([1, 128], FP16)
            nc.vector.memset(ones_h[:], 1.0)

            # phase-1 outputs, one tile per (head, q-chunk) so phase-2
            # readers never pick up false deps on later chunks' writes
            qt_sb = [[res.tile([128, QC], BF16, tag=f"qt{h}_{c}", name=f"qt{h}_{c}")
                      for c in range(NQC)] for h in range(HQL)]
            kt_sb = [[res.tile([128, QC], BF16, tag=f"kt{h}_{c}", name=f"kt{h}_{c}")
                      for c in range(NQC)] for h in range(HKL)]
            v_sb = [res.tile([128, 4, HKL * HD], BF16, tag=f"v{c}", name=f"v{c}")
                    for c in range(NQC)]
            ot_sb = [[res.tile([128, QC], BF16, tag=f"ot{h}_{c}", name=f"ot{h}_{c}")
                      for c in range(NQC)] for h in range(HQL)]

            rope_q = []               # deferred qc-3 rope closures

            # ---------- phase 1: QKV projection + RoPE ----------
            with tc.tile_pool(name="ps1", bufs=1, space="PSUM") as ps1:
                for qc in range(NQC):
                    qsl = slice(qc * QC, (qc + 1) * QC)
                    qps = [ps1.tile([128, QC], F32, tag=f"qps{h}", name=f"qps{h}_{qc}") for h in range(HQL)]
                    kps = [ps1.tile([128, QC], F32, tag=f"kps{h}", name=f"kps{h}_{qc}") for h in range(HKL)]
                    vps = ps1.tile([128, 4, HKL * HD], F32, tag="vps")
                    # smaller first batches so the first matmuls start early
                    groups = [(0, 1), (1, 1), (2, 2), (4, 4), (8, 4), (12, 4)] if qc == 0 \
                        else [(0, 4), (4, 4), (8, 4), (12, 4)]
                    gstarts = {a: n for a, n in groups}
                    for kt in range(NKT):
                        if kt in gstarts:
                            n = gstarts[kt]
                            g = slice(kt, kt + n)
                            if qc == 0:
                                # batched weight loads on the scalar HWDGE
                                # queue (a DMA descriptor costs ~700ns issue)
                                nc.scalar.dma_start(
                                    wq_sb[:, g, :], wq_d[g].rearrange("k p m -> p k m"))
                                nc.scalar.dma_start(
                                    wk_sb[:, g, :], wk_d[g].rearrange("k p m -> p k m"))
                                nc.scalar.dma_start(
                                    wv_sb[:, g, :], wv_d[g].rearrange("k p m -> p k m"))
                            xt4 = xpool.tile([128, 4, QC], BF16, tag="xt",
                                             name=f"x{qc}_{kt}")
                            nc.sync.dma_start(
                                xt4[:, 0:n, :], xT_d[g, :, qsl].rearrange("k p m -> p k m"))
                            xbase = kt
                            for _ in range(2):
                                if rope_q:
                                    rope_q.pop(0)()
                        if qc == 0 and kt == 13:
                            nc.scalar.dma_start(cos_sb[:], cos_d[:])
                            nc.scalar.dma_start(sin_sb[:], sin_d[:])
                        if qc == 0 and kt == 15:
                            nc.scalar.dma_start(tri_sb[:], tri_d[:])
                        if qc == 1 and kt == 12:
                            # wo is first read ~30us into phase 2; loading it
                            # here keeps it out of the congested startup window
                            nc.scalar.dma_start(wo_sb[:],
                                                wo_d.rearrange("h p m -> p h m"))
                        xt = xt4[:, kt - xbase, :]
                        st, sp = (kt == 0), (kt == NKT - 1)
                        for h in range(HQL):
                            nc.tensor.matmul(qps[h][:], wq_sb[:, kt, h * HD:(h + 1) * HD],
                                             xt, start=st, stop=sp)
                        for h in range(HKL):
                            nc.tensor.matmul(kps[h][:], wk_sb[:, kt, h * HD:(h + 1) * HD],
                                             xt, start=st, stop=sp)
                        for sb in range(4):
                            # two 256-col outputs share one PSUM bank: only the
                            # bank's first writer may clear has_written (start)
                            nc.tensor.matmul(vps[:, sb, :], xt4[:, kt - xbase, sb * 128:(sb + 1) * 128],
                                             wv_sb[:, kt, :],
                                             start=(st and sb % 2 == 0), stop=sp,
                                             skip_group_check=True)

                    if qc == 0:
                        # warm the rope tables on DVE so rope TTs carry a
                        # single cross-engine wait (the evac'd tile)
                        wmA = small.tile([1, 1], BF16, tag="warmA")
                        nc.vector.tensor_copy(wmA[:], cos_sb[0:1, 0:1])
                        wmB = small.tile([1, 1], BF16, tag="warmB")
                        nc.vector.tensor_copy(wmB[:], sin_sb[0:1, 0:1])
                        wmC = small.tile([1, 1], BF16, tag="warmC")
                        nc.vector.tensor_copy(wmC[:], tri_sb[0:1, 0:1])

                    # drain: evac copies split ACT/DVE in PE-consumption
                    # order so the 8 psum banks free fast; RoPE follows on DVE
                    evacs = []
                    srcs = [qps[0], qps[1], qps[2], qps[3], kps[0], kps[1]]
                    dsts = [qt_sb[0][qc], qt_sb[1][qc], qt_sb[2][qc],
                            qt_sb[3][qc], kt_sb[0][qc], kt_sb[1][qc]]
                    act_set = (0, 1, 5)     # ACT: q0, q1, k1 (then v)
                    for i, (ps, dst) in enumerate(zip(srcs, dsts)):
                        qsb = evpool.tile([128, QC], BF16, tag="evac",
                                          name=f"evac{qc}_{i}")
                        if i in act_set:
                            nc.scalar.copy(qsb[:], ps[:])
                        else:
                            nc.vector.tensor_copy(qsb[:], ps[:])
                        evacs.append((qsb, dst))
                    nc.scalar.copy(
                        v_sb[qc][:].rearrange("p a b -> p (a b)"),
                        vps[:].rearrange("p a b -> p (a b)"))

                    def rope_one(qsb, dst, qc=qc, i=0):
                        qsw = rtmp.tile([128, QC], BF16, tag="swap",
                                        name=f"swap{qc}_{i}")
                        nc.vector.tensor_copy(qsw[0:64, :], qsb[64:128, :])
                        nc.vector.tensor_copy(qsw[64:128, :], qsb[0:64, :])
                        a_t = rtmp.tile([128, QC], BF16, tag="ropeA")
                        nc.vector.tensor_tensor(a_t[:], qsb[:],
                                                cos_sb[:, qc * QC:(qc + 1) * QC],
                                                OP.mult)
                        b_t = rtmp.tile([128, QC], BF16, tag="ropeB")
                        nc.vector.tensor_tensor(b_t[:], qsw[:],
                                                sin_sb[:, qc * QC:(qc + 1) * QC],
                                                OP.mult)
                        nc.vector.tensor_tensor(dst[:], a_t[:], b_t[:], OP.add)

                    # defer this chunk's rope into the next chunk's kt
                    # loop (last chunk: into phase 2) so qc-boundary evacs
                    # never queue behind 8us of rope on the DVE
                    for i, (qsb, dst) in enumerate(evacs):
                        rope_q.append(
                            lambda qsb=qsb, dst=dst, qc=qc, i=i:
                            rope_one(qsb, dst, qc, i))

            # ---------- phase 2+3: attention with interleaved output proj ----
            with (
                tc.tile_pool(name="ps_s", bufs=2, space="PSUM") as ps_s,
                tc.tile_pool(name="ps_o", bufs=2, space="PSUM") as ps_o,
                tc.tile_pool(name="ps_z", bufs=1, space="PSUM") as ps_z,
                tc.tile_pool(name="ps_r", bufs=1, space="PSUM") as ps_r,
                tc.tile_pool(name="ps_f", bufs=2, space="PSUM") as ps_f,
            ):
                pending = [None]          # deferred rb-broadcast + normalize
                chunks = []               # deferred out-proj (sb, dcp) pairs
                qn = [0]                  # out-DMA queue alternator

                def emit_chunk(sb, dcp, tail=0):
                    o2 = outp.tile([128, 2, QC], BF16, tag="osb",
                                   name=f"o{sb}_{dcp}")
                    for j in range(2):
                        dc = 2 * dcp + j
                        fps = ps_f.tile([128, QC], F32, tag="fps",
                                        name=f"fps{sb}_{dc}")
                        for h in range(HQL):
                            nc.tensor.matmul(
                                fps[:], ot_sb[h][sb // 4][:, (sb % 4) * 128:(sb % 4 + 1) * 128],
                                wo_sb[:, h, dc * QC:(dc + 1) * QC],
                                start=(h == 0), stop=(h == HQL - 1))
                        if j == 0 or tail:
                            nc.scalar.copy(o2[:, j, :], fps[:])
                        else:
                            nc.vector.tensor_copy(o2[:, j, :], fps[:])
                    eng = nc.sync if qn[0] % 2 == 0 else nc.scalar
                    qn[0] += 1
                    eng.dma_start(out_d[sb, :, dcp * 2 * QC:(dcp + 1) * 2 * QC],
                                  o2[:].rearrange("p a b -> p (a b)"))

                def flush_pending():
                    if pending[0] is not None:
                        pending[0]()
                        pending[0] = None

                # process q-chunks deepest-first-ish: the 4-kblock
                # qc=0 heads are chain-latency-bound, so run them LAST when
                # out-proj chunks exist to fill the PE bubbles
                for qc in (2, 3, 1, 0):
                    nkb = 4 * qc + 4
                    inj = 4 if nkb > 4 else 2   # chunk-injection stride
                    for h in range(HQL):
                        kv = h // 2
                        ops_t = ps_o.tile([128, QC], F32, tag="opv",
                                          name=f"opv{qc}_{h}")
                        zps_t = ps_z.tile([1, QC], F32, tag="zps",
                                          name=f"zps{qc}_{h}")
                        for kb in range(nkb):
                            off = (kb - 4 * qc) * 128 if kb >= 4 * qc else 0
                            sps = ps_s.tile([128, QC], F32, tag="sps",
                                            name=f"sps{qc}_{h}_{kb}")
                            nc.tensor.matmul(
                                sps[:, off:],
                                kt_sb[kv][kb // 4][:, (kb % 4) * 128:(kb % 4 + 1) * 128],
                                qt_sb[h][qc][:, off:],
                                start=True, stop=True)
                            e_t = epool.tile([128, QC], BF16, tag="etile",
                                             name=f"e{qc}_{h}_{kb}")
                            nc.scalar.activation(e_t[:, off:], sps[:, off:],
                                                 AF.Exp, scale=SCALE)
                            if kb >= 4 * qc:
                                nc.vector.tensor_tensor(
                                    e_t[:, off:off + 128], e_t[:, off:off + 128],
                                    tri_sb[:], OP.mult)
                            if kb == 1:
                                flush_pending()
                            st, sp = (kb == 0), (kb == nkb - 1)
                            nc.tensor.matmul(
                                ops_t[:, off:],
                                v_sb[kb // 4][:, kb % 4, kv * HD:(kv + 1) * HD],
                                e_t[:, off:], start=st, stop=sp,
                                skip_group_check=True)
                            nc.tensor.matmul(
                                zps_t[:, off:], ones_bf[:], e_t[:, off:],
                                start=st, stop=sp, skip_group_check=True)
                            if kb % inj == inj - 1 and chunks:
                                emit_chunk(*chunks.pop(0))

                        # deferred qc-3 rope: two tiles per head keeps
                        # the DVE queue shallow and finishes all 6 before
                        # qc-3 attention (processed second)
                        for _ in range(2):
                            if rope_q:
                                rope_q.pop(0)()

                        # reciprocal chain runs on DVE right away; the
                        # rb-broadcast matmul + normalize TT are deferred so
                        # the PE queue never waits on this chain
                        z_sb = small.tile([1, QC], F32, tag="zsb")
                        nc.vector.tensor_copy(z_sb[:], zps_t[:])
                        rz = small.tile([1, QC], F32, tag="rz")
                        nc.vector.reciprocal_approx_fast(rz[:], z_sb[:])
                        rz_h = small.tile([1, QC], FP16, tag="rzh")
                        nc.vector.tensor_copy(rz_h[:], rz[:])

                        def make_norm(h=h, qc=qc, ops_t=ops_t, rz_h=rz_h):
                            def norm():
                                rb_ps = ps_r.tile([128, QC], F32, tag="rbps",
                                                  name=f"rb{qc}_{h}")
                                nc.tensor.matmul(rb_ps[:], ones_h[:], rz_h[:],
                                                 start=True, stop=True)
                                rb_sb = small.tile([128, QC], F32, tag="rbsb",
                                                   name=f"rbs{qc}_{h}")
                                nc.vector.tensor_copy(rb_sb[:], rb_ps[:])
                                nc.vector.tensor_tensor(
                                    ot_sb[h][qc][:], ops_t[:], rb_sb[:],
                                    OP.mult)
                                if h == HQL - 1:
                                    for sb in range(4 * qc, 4 * qc + 4):
                                        for dcp in range(2):
                                            chunks.append((sb, dcp))
                            return norm

                        flush_pending()   # at most one outstanding
                        pending[0] = make_norm()

                flush_pending()
                while rope_q:
                    rope_q.pop(0)()
                ti = 0
                while chunks:
                    ti += 1
                    emit_chunk(*chunks.pop(0), tail=ti)

    nc.compile()
    return nc


_NC_CACHE = None


def _get_nc():
    global _NC_CACHE
    if _NC_CACHE is None:
        _NC_CACHE = _build_nc()
    return _NC_CACHE


def _rope_tables():
    inv = 1.0 / (ROPE_THETA ** (np.arange(0, HD, 2, dtype=np.float64) / HD))  # [64]
    t = np.arange(S, dtype=np.float64)
    ang = np.outer(inv, t)                      # [64, S]
    cos = np.cos(ang).astype(np.float32)
    sin = np.sin(ang).astype(np.float32)
    cos128 = np.concatenate([cos, cos], axis=0).astype(BF)  # [128, S]
    sinM = np.concatenate([-sin, sin], axis=0).astype(BF)
    return cos128, sinM


def prepare_inputs(x, wq, wk, wv, wo):
    """Build the 8 per-core input dicts from full inputs."""
    perm = np.concatenate([np.arange(0, HD, 2), np.arange(1, HD, 2)])
    cos128, sinM = _rope_tables()
    tri = np.greater_equal.outer(np.arange(128), np.arange(128)).T.astype(BF)

    x = np.asarray(x, np.float32)
    wq = np.asarray(wq, np.float32).reshape(HQ, HD, D)[:, perm, :]
    wk = np.asarray(wk, np.float32).reshape(HKV, HD, D)[:, perm, :]
    wv = np.asarray(wv, np.float32).reshape(HKV, HD, D)
    wo = np.asarray(wo, np.float32)              # [D, HQ*HD]

    in_maps = []
    for c in range(NCORES):
        b, hg = divmod(c, TP)
        qh = slice(hg * HQL, (hg + 1) * HQL)
        kh = slice(hg * HKL, (hg + 1) * HKL)
        xT = np.ascontiguousarray(x[b].T).astype(BF).reshape(NKT, 128, S)
        wq_t = np.ascontiguousarray(
            wq[qh].reshape(HQL * HD, D).T).astype(BF).reshape(NKT, 128, HQL * HD)
        wk_t = np.ascontiguousarray(
            wk[kh].reshape(HKL * HD, D).T).astype(BF).reshape(NKT, 128, HKL * HD)
        wv_t = np.ascontiguousarray(
            wv[kh].reshape(HKL * HD, D).T).astype(BF).reshape(NKT, 128, HKL * HD)
        wo_t = np.ascontiguousarray(
            wo[:, hg * HQL * HD:(hg + 1) * HQL * HD].T.reshape(HQL, HD, D)
        ).astype(BF)
        in_maps.append({
            "xT": xT, "wq_t": wq_t, "wk_t": wk_t, "wv_t": wv_t, "wo_t": wo_t,
            "cos128": cos128, "sinM": sinM, "tri": tri,
        })
    return in_maps


def _install_ntff_hook():
    """The agent image's antenv lacks axon_hooks; synthesize it so
    run_bass_kernel_spmd(trace=True) can capture NTFF profiles."""
    import sys as _sys
    import types, contextlib, ctypes

    if "antenv.axon_hooks" in _sys.modules:
        return
    so_path = "/opt/axon/libaxon_pjrt.so"
    lib = ctypes.CDLL(so_path)
    if not hasattr(lib, "axon_start_nrt_profile"):
        return
    lib.axon_start_nrt_profile.argtypes = [ctypes.POINTER(ctypes.c_int64),
                                           ctypes.c_size_t]
    lib.axon_start_nrt_profile.restype = ctypes.c_int64
    lib.axon_stop_nrt_profile.argtypes = [ctypes.c_char_p]
    lib.axon_stop_nrt_profile.restype = ctypes.c_int64

    @contextlib.contextmanager
    def _hook(output_dir, device_ids):
        import jax
        jax.devices()
        if device_ids:
            ids = (ctypes.c_int64 * len(device_ids))(*device_ids)
            rc = lib.axon_start_nrt_profile(ids, len(device_ids))
        else:
            rc = lib.axon_start_nrt_profile(None, 0)
        if rc != 0:
            raise RuntimeError(f"axon_start_nrt_profile rc={rc}")
        try:
            yield
        finally:
            n = lib.axon_stop_nrt_profile(str(output_dir).encode())
            print(f"ntff profile: {n} file(s) written to {output_dir}",
                  file=_sys.stderr)

    mod = types.ModuleType("antenv.axon_hooks")
    mod.get_axon_ntff_profile_hook = lambda: _hook
    mod.set_axon_ntff_profile_hook = lambda h: None
    _sys.modules["antenv.axon_hooks"] = mod
    try:
        import antenv
        antenv.axon_hooks = mod
    except ImportError:
        pass


def kernel(x, wq, wk, wv, wo, _trace=False, _trace_cores=None):
    in_maps = prepare_inputs(x, wq, wk, wv, wo)
    if _trace:
        _install_ntff_hook()
    nc = _get_nc()
    res = run_bass_kernel_spmd(
        nc, in_maps, core_ids=list(range(NCORES)),
        trace=_trace, trace_cores=_trace_cores)
    out = np.zeros((B, S, D), np.float32)
    for c in range(NCORES):
        b = c // TP
        out[b] += np.asarray(res.results[c]["out"], np.float32).reshape(S, D)
    kernel.last_results = res
    return out


if __name__ == "__main__":
    rng = np.random.default_rng(0)
    x = rng.standard_normal((B, S, D), dtype=np.float32)
    sc = 1.0 / np.sqrt(D)
    wq = (rng.standard_normal((HQ * HD, D), dtype=np.float32) * sc)
    wk = (rng.standard_normal((HKV * HD, D), dtype=np.float32) * sc)
    wv = (rng.standard_normal((HKV * HD, D), dtype=np.float32) * sc)
    wo = (rng.standard_normal((D, HQ * HD), dtype=np.float32) * sc)
    out = kernel(x, wq, wk, wv, wo)
    print("ran", out.shape, out.dtype, float(np.abs(out).mean()))
